# revision 36
# baseline (speedup 1.0000x reference)
"""Trainium2 Bass kernel for nn_AttentionHead.

Computation (per batch b):
    Q = Wq @ x_b, K = Wk @ x_b, V = Wv @ x_b        (x_b: [C=256, N=4096])
    S = Q^T K   [N, N];  A = softmax_k(S)
    out_b = V @ A^T                                  ([VC=128, N])

Sharding: 8 cores = 4 batches x 2 query-halves. Each core computes K/V^T for
its full batch and Q for its 2048-query half; a flash-style loop over 32 key
chunks of 128 never materializes the full [4096, 4096] affinity.

Numerics: the host casts x and the weights to fp16 (halves input DMA and
runs every matmul at the full-rate 16-bit PE path while keeping ~10 mantissa
bits through the logits, accumulated in fp32 PSUM). exp tiles are bf16 (fp16
would overflow: logits reach ~19 un-normalized). Softmax denominators: exp
tiles are summed on DVE in 2048-wide grouped chains down to 5 partials per
query-half; the final cross-partition reduction and the normalization happen
on the host during unshard.
"""

import numpy as np

B, C, VC, H, W = 4, 256, 128, 64, 64
N = H * W            # keys per batch
MQ = N // 2          # queries per core
QT = 1024            # query tile (PSUM-sized)
KC = N // 128        # key chunks of 128
NG = 5               # softmax partial-sum tiles per query tile

_cached_nc = None


def _build():
    from contextlib import ExitStack

    import concourse.bacc as bacc
    import concourse.mybir as mybir
    import concourse.tile as tile

    f32 = mybir.dt.float32
    f16 = mybir.dt.float16
    bf16 = mybir.dt.bfloat16
    Exp = mybir.ActivationFunctionType.Exp

    nc = bacc.Bacc("TRN2", target_bir_lowering=False, debug=False, num_devices=8)

    # Inputs are pre-packed on the host so every DMA source is a fully
    # contiguous [128, cols] block (sequential HBM reads, max DMA rate):
    #   wz: both C-halves of [Wq^T | Wk^T | Wv^T]
    #   xq_p: query cols as 4 blocks (2 col-halves x 2 C-halves)
    #   xk_p1: key cols 0:1024 as 4 blocks of 512
    #   xk_p2: key cols 1024:4096 as 6 blocks of 1024 (2 C-halves each)
    wz_d = nc.dram_tensor("wz", [2, 128, 3 * VC], f16, kind="ExternalInput")
    xq_d = nc.dram_tensor("xq", [4, 128, 512], f16, kind="ExternalInput")
    xq2_d = nc.dram_tensor("xq2", [2, 128, 1024], f16, kind="ExternalInput")
    xk1_d = nc.dram_tensor("xk1", [4, 128, 512], f16, kind="ExternalInput")
    xk2_d = nc.dram_tensor("xk2", [6, 128, 1024], f16, kind="ExternalInput")
    oc_d = nc.dram_tensor("oc", [2, 128, QT], f32, kind="ExternalOutput")
    oss_d = nc.dram_tensor("oss", [2, NG, 128, QT], bf16, kind="ExternalOutput")

    with tile.TileContext(nc) as tc, ExitStack() as ctx:
        persist = ctx.enter_context(tc.tile_pool(name="persist", bufs=1))
        wpool = ctx.enter_context(tc.tile_pool(name="w", bufs=1))
        xp = ctx.enter_context(tc.tile_pool(name="xp", bufs=1))

        wall_t = [
            wpool.tile([128, 3 * VC], f16, tag=f"wall{cc}", name=f"wall{cc}")
            for cc in range(2)
        ]
        _woff = {"wq": 0, "wk": VC, "wv": 2 * VC}
        wts = {
            (nm, cc): wall_t[cc][:, off : off + VC]
            for nm, off in _woff.items()
            for cc in range(2)
        }

        K_t = persist.tile([128, N], f16, tag="K")
        Q_t = persist.tile([128, MQ], f16, tag="Q")
        VT = persist.tile([128, KC * 128], bf16, tag="VT")

        xk_t = [
            xp.tile([128, N], f16, tag=f"xk{cc}", name=f"xk{cc}") for cc in range(2)
        ]
        xq_t = [
            xp.tile([128, MQ], f16, tag=f"xq{cc}", name=f"xq{cc}") for cc in range(2)
        ]

        # Input DMA: each packed block is striped cc0->sync / cc1->gpsimd so
        # both halves complete together at the combined HBM rate, in
        # consumption order. Per-engine dependency windows bound in-flight
        # pieces so early pieces finish early.
        from concourse.tile_rust import add_dep_helper

        _dmas = {}
        _eng = ((nc.sync, "s"), (nc.gpsimd, "g"))

        def _issue(cc, dst_ap, src_ap):
            eng, ename = _eng[cc]
            lst = _dmas.setdefault(ename, [])
            ins = eng.dma_start(dst_ap, src_ap)
            if len(lst) >= 4:
                add_dep_helper(ins.ins, lst[-4].ins, reason="dma window")
            lst.append(ins)

        for cc in range(2):
            _issue(cc, wall_t[cc][:], wz_d[cc])
        for cc in range(2):
            _issue(cc, xq_t[cc][:, 0:512], xq_d[cc])
        for cc in range(2):
            _issue(cc, xk_t[cc][:, 0:512], xk1_d[cc])
        for cc in range(2):
            _issue(cc, xq_t[cc][:, 512:1024], xq_d[2 + cc])
        for cc in range(2):
            _issue(cc, xk_t[cc][:, 512:1024], xk1_d[2 + cc])
        for blk in range(3):
            for cc in range(2):
                _issue(
                    cc,
                    xk_t[cc][:, 1024 + blk * 1024 : 2048 + blk * 1024],
                    xk2_d[2 * blk + cc],
                )
        for cc in range(2):
            _issue(cc, xq_t[cc][:, 1024:2048], xq2_d[cc])

        def emit_proj_tile(pool, dst, wnm, xt, t, copy_eng=None):
            ps = pool.tile([128, 512], f32, tag="projps", name="ps")
            for cc in range(2):
                nc.tensor.matmul(
                    ps[:],
                    wts[(wnm, cc)][:],
                    xt[cc][:, t * 512 : (t + 1) * 512],
                    start=(cc == 0),
                    stop=(cc == 1),
                )
            dstap = dst[:, t * 512 : (t + 1) * 512]
            if copy_eng is None:
                nc.vector.tensor_copy(dstap, ps[:])
            else:
                copy_eng.copy(dstap, ps[:])

        def emit_vt_quad(pool, q, copy_eng=None):
            # V^T blocks 4q..4q+3: each [n=128, d=128] = x_block.T @ Wv.T;
            # four blocks share one PSUM tile so one wide copy drains them.
            tp = pool.tile([128, 512], f32, tag="projps", name="tp")
            for jj in range(4):
                j = 4 * q + jj
                for cc in range(2):
                    nc.tensor.matmul(
                        tp[:, jj * 128 : (jj + 1) * 128],
                        xk_t[cc][:, j * 128 : (j + 1) * 128],
                        wts[("wv", cc)][:],
                        start=(cc == 0),
                        stop=(cc == 1),
                    )
            if copy_eng is None:
                nc.vector.tensor_copy(VT[:, q * 512 : (q + 1) * 512], tp[:])
            else:
                copy_eng.copy(VT[:, q * 512 : (q + 1) * 512], tp[:])

        # Floors (ms): don't emit projection work before its DMA piece can
        # have landed (input streams at ~0.24 MB/us from ~7.6us), so the
        # in-order PE queue never blocks on a DMA semaphore.
        QF0, KF0, QF1 = 0.0094, 0.0105, 0.0115
        K_FLOOR = {1: 0.0128, 2: 0.0148, 3: 0.0152, 4: 0.0168, 5: 0.0172,
                   6: 0.0188, 7: 0.0192}
        V_FLOOR = {0: 0.0120, 1: 0.0130, 2: 0.0150, 3: 0.0154, 4: 0.0170,
                   5: 0.0174, 6: 0.0190, 7: 0.0194}
        XQ1 = 0.0208

        spool = ctx.enter_context(tc.tile_pool(name="spool", bufs=2, space="PSUM"))
        pcpool = ctx.enter_context(tc.tile_pool(name="pcpool", bufs=1, space="PSUM"))

        with tc.tile_pool(name="projps", bufs=2, space="PSUM") as pps:
            # K/VT copies ride the idle ACT engine pre-first-exp so the
            # QK(0,0) critical path never waits behind DVE copy order.
            with tc.tile_wait_until(QF0):
                emit_proj_tile(pps, Q_t, "wq", xq_t, 0)
            with tc.tile_wait_until(KF0):
                emit_proj_tile(pps, K_t, "wk", xk_t, 0, copy_eng=nc.scalar)
            with tc.tile_wait_until(QF1):
                emit_proj_tile(pps, Q_t, "wq", xq_t, 1)

        with (
            tc.tile_pool(name="lzps", bufs=2, space="PSUM") as lzps,
            tc.tile_pool(name="epool", bufs=4) as epool,
            tc.tile_pool(name="treep", bufs=2) as treep,
            tc.tile_pool(name="fold", bufs=3) as foldp,
            tc.tile_pool(name="opool", bufs=2) as opool,
        ):
            pairs = [(qt, j) for qt in range(2) for j in range(KC)]
            ps_tiles = {}

            def emit_qk(qt, j):
                ps = spool.tile([128, QT], f32, tag="ps", name="ps")
                for qq in range(2):
                    nc.tensor.matmul(
                        ps[:, qq * 512 : (qq + 1) * 512],
                        K_t[:, j * 128 : (j + 1) * 128],
                        Q_t[:, qt * QT + qq * 512 : qt * QT + (qq + 1) * 512],
                        start=True,
                        stop=True,
                    )
                ps_tiles[(qt, j)] = ps

            pc = None
            es_dt = None
            acc = None
            nfold = None
            emit_qk(*pairs[0])
            for i, (qt, j) in enumerate(pairs):
                if i + 1 < len(pairs):
                    emit_qk(*pairs[i + 1])
                if j == 0:
                    pc = pcpool.tile([128, QT], f32, tag="pc", name="pc")
                ps = ps_tiles.pop((qt, j))
                if j % 2 == 0:
                    es_dt = epool.tile([128, 2 * QT], bf16, tag="es", name="es")
                es = es_dt[:, (j % 2) * QT : (j % 2 + 1) * QT]
                nc.scalar.activation(es, ps[:], Exp)
                # interleave remaining projections into the first pass,
                # one per two iterations so the PE never starves ACT (each
                # K tile t / VT quad q is consumed from iteration 4t / 4q)
                if qt == 0 and j % 2 == 1 and j <= 27:
                    s = (j + 1) // 2       # 1..14
                    if s == 0:
                        pass
                    elif s % 2 == 1:
                        t = (s + 1) // 2   # K tiles 1..7 at j=1,5,...
                        with tc.tile_wait_until(K_FLOOR[t]):
                            emit_proj_tile(lzps, K_t, "wk", xk_t, t)
                    else:
                        q = s // 2         # VT quads 1..7 at j=3,7,...
                        with tc.tile_wait_until(V_FLOOR[q]):
                            emit_vt_quad(lzps, q)
                if qt == 0 and j == 0:
                    with tc.tile_wait_until(V_FLOOR[0]):
                        emit_vt_quad(lzps, 0, copy_eng=nc.scalar)
                if qt == 0 and j in (26, 28):
                    with tc.tile_wait_until(XQ1):
                        emit_proj_tile(lzps, Q_t, "wq", xq_t, 2 + (j - 26) // 2)
                first, last = j == 0, j == KC - 1
                for qq in range(2):
                    sl = slice(qq * 512, (qq + 1) * 512)
                    nc.tensor.matmul(
                        pc[:, sl],
                        VT[:, j * 128 : (j + 1) * 128],
                        es[:, qq * 512 : (qq + 1) * 512],
                        start=first,
                        stop=last,
                    )
                # Softmax partial sums on DVE. Groups 0-2 (j 0..23): wide
                # [128, 2048] chained adds over 4 double-tiles, then one fold
                # to [128, QT]. Group 3 (j 24..31): per-double-tile narrow
                # folds chained, with the last fold shipped as its own
                # partial so the post-last-exp tail is just fold+DMA.
                if j % 2 == 1:
                    g, m = j // 8, (j % 8) // 2
                    if g < 3:
                        if m == 0:
                            acc = es_dt
                        else:
                            if m == 1:
                                nacc = treep.tile(
                                    [128, 2 * QT], bf16, tag="acc", name="acc"
                                )
                                nc.vector.tensor_add(nacc[:], acc[:], es_dt[:])
                                acc = nacc
                            else:
                                nc.vector.tensor_add(acc[:], acc[:], es_dt[:])
                            if m == 3:
                                fo = foldp.tile(
                                    [128, QT], bf16, tag="fo", name="fo"
                                )
                                nc.vector.tensor_add(
                                    fo[:], acc[:, 0:QT], acc[:, QT : 2 * QT]
                                )
                                nc.sync.dma_start(oss_d[qt, g], fo[:])
                    else:
                        fo = foldp.tile([128, QT], bf16, tag="fo", name="fo")
                        nc.vector.tensor_add(
                            fo[:], es_dt[:, 0:QT], es_dt[:, QT : 2 * QT]
                        )
                        if m == 0:
                            nfold = fo
                        elif m < 3:
                            nc.vector.tensor_add(nfold[:], nfold[:], fo[:])
                            if m == 2:
                                nc.sync.dma_start(oss_d[qt, 3], nfold[:])
                        else:
                            nc.sync.dma_start(oss_d[qt, 4], fo[:])
                if last:
                    # oc drain: qt=0 copies on DVE (slack mid-run); qt=1
                    # copies on ACT, which is idle once its last exp is done,
                    # so the tail never serializes with the DVE fold chain.
                    # The DMA rides gpsimd's queue, away from sync's oss DMAs.
                    so = opool.tile([128, QT], f32, tag="so", name="so")
                    for qq in range(2):
                        sl = slice(qq * 512, (qq + 1) * 512)
                        if qt == 0:
                            nc.vector.tensor_copy(so[:, sl], pc[:, sl])
                        else:
                            nc.scalar.copy(so[:, sl], pc[:, sl])
                        eng = nc.gpsimd if qq == 0 else nc.sync
                        eng.dma_start(oc_d[qt, :, sl], so[:, sl])

    nc.compile()
    return nc


def make_in_maps(x, Wq, Wk, Wv):
    x = np.asarray(x, dtype=np.float32).reshape(B, C, N).astype(np.float16)
    wz = np.concatenate(
        [np.asarray(w, np.float32).T for w in (Wq, Wk, Wv)], axis=1
    ).astype(np.float16)
    wt = {"wz": np.ascontiguousarray(wz.reshape(2, 128, 3 * VC))}

    def blocks(m, spans):
        # pack [cc-half, col-span] blocks of a [C, *] matrix contiguously
        return np.stack(
            [m[cc * 128 : (cc + 1) * 128, c0:c1] for c0, c1 in spans for cc in (0, 1)]
        )

    in_maps = []
    for core in range(8):
        b, h = core // 2, core % 2
        xc = x[b]
        xq = xc[:, h * MQ : (h + 1) * MQ]
        in_maps.append(
            {
                "xq": blocks(xq, [(0, 512), (512, 1024)]),
                "xq2": blocks(xq, [(1024, 2048)]),
                "xk1": blocks(xc, [(0, 512), (512, 1024)]),
                "xk2": blocks(
                    xc, [(1024, 2048), (2048, 3072), (3072, 4096)]
                ),
                **wt,
            }
        )
    return in_maps


def assemble_output(results):
    out = np.empty((B, VC, N), dtype=np.float32)
    for core, r in enumerate(results):
        b, h = core // 2, core % 2
        # oss: [2, NG, 128, QT] partial sums; reduce groups+partitions
        sums = r["oss"].astype(np.float32).sum(axis=(1, 2))[:, None, :]  # [2,1,QT]
        core_out = r["oc"] / sums                                        # [2,128,QT]
        out[b, :, h * MQ : (h + 1) * MQ] = np.concatenate(
            [core_out[0], core_out[1]], axis=1
        )
    return out.reshape(B, VC, H, W)


def _results_sane(results):
    for r in results:
        oc, oss = r["oc"], np.asarray(r["oss"], dtype=np.float32)
        if not (np.isfinite(oc).all() and np.isfinite(oss).all()):
            return False
        if oss.sum(axis=(1, 2)).min() <= 0.0:      # softmax denominators
            return False
    return True


def kernel(x, Wq, Wk, Wv):
    global _cached_nc
    from concourse.bass_utils import run_bass_kernel_spmd

    if _cached_nc is None:
        _cached_nc = _build()
    in_maps = make_in_maps(x, Wq, Wk, Wv)
    results = None
    for attempt in range(3):
        try:
            res = run_bass_kernel_spmd(
                _cached_nc, in_maps, core_ids=list(range(8))
            )
        except Exception:
            if attempt == 2:
                raise
            continue
        results = res.results
        if _results_sane(results):
            break
    return assemble_output(results)


# revision 38
# speedup vs baseline: 1.0090x; 1.0090x over previous
"""Trainium2 Bass kernel for nn_AttentionHead.

Computation (per batch b):
    Q = Wq @ x_b, K = Wk @ x_b, V = Wv @ x_b        (x_b: [C=256, N=4096])
    S = Q^T K   [N, N];  A = softmax_k(S)
    out_b = V @ A^T                                  ([VC=128, N])

Sharding: 8 cores = 4 batches x 2 query-halves. Each core computes K/V^T for
its full batch and Q for its 2048-query half; a flash-style loop over 32 key
chunks of 128 never materializes the full [4096, 4096] affinity.

Numerics: the host casts x and the weights to fp16 (halves input DMA and
runs every matmul at the full-rate 16-bit PE path while keeping ~10 mantissa
bits through the logits, accumulated in fp32 PSUM). exp tiles are bf16 (fp16
would overflow: logits reach ~19 un-normalized). Softmax denominators: exp
tiles are summed on DVE in 2048-wide grouped chains down to 5 partials per
query-half; the final cross-partition reduction and the normalization happen
on the host during unshard.
"""

import numpy as np

B, C, VC, H, W = 4, 256, 128, 64, 64
N = H * W            # keys per batch
MQ = N // 2          # queries per core
QT = 1024            # query tile (PSUM-sized)
KC = N // 128        # key chunks of 128
NG = 5               # softmax partial-sum tiles per query tile

_cached_nc = None


def _build():
    from contextlib import ExitStack

    import concourse.bacc as bacc
    import concourse.mybir as mybir
    import concourse.tile as tile

    f32 = mybir.dt.float32
    f16 = mybir.dt.float16
    bf16 = mybir.dt.bfloat16
    Exp = mybir.ActivationFunctionType.Exp

    nc = bacc.Bacc("TRN2", target_bir_lowering=False, debug=False, num_devices=8)

    # Inputs are pre-packed on the host so every DMA source is a fully
    # contiguous [128, cols] block (sequential HBM reads, max DMA rate):
    #   wz: both C-halves of [Wq^T | Wk^T | Wv^T]
    #   xq_p: query cols as 4 blocks (2 col-halves x 2 C-halves)
    #   xk_p1: key cols 0:1024 as 4 blocks of 512
    #   xk_p2: key cols 1024:4096 as 6 blocks of 1024 (2 C-halves each)
    wz_d = nc.dram_tensor("wz", [2, 128, 3 * VC], f16, kind="ExternalInput")
    xq_d = nc.dram_tensor("xq", [4, 128, 512], f16, kind="ExternalInput")
    xq2_d = nc.dram_tensor("xq2", [2, 128, 1024], f16, kind="ExternalInput")
    xk1_d = nc.dram_tensor("xk1", [4, 128, 512], f16, kind="ExternalInput")
    xk2_d = nc.dram_tensor("xk2", [6, 128, 1024], f16, kind="ExternalInput")
    oc_d = nc.dram_tensor("oc", [2, 128, QT], f32, kind="ExternalOutput")
    oss_d = nc.dram_tensor("oss", [2, NG, 128, QT], bf16, kind="ExternalOutput")

    with tile.TileContext(nc) as tc, ExitStack() as ctx:
        persist = ctx.enter_context(tc.tile_pool(name="persist", bufs=1))
        wpool = ctx.enter_context(tc.tile_pool(name="w", bufs=1))
        xp = ctx.enter_context(tc.tile_pool(name="xp", bufs=1))

        wall_t = [
            wpool.tile([128, 3 * VC], f16, tag=f"wall{cc}", name=f"wall{cc}")
            for cc in range(2)
        ]
        _woff = {"wq": 0, "wk": VC, "wv": 2 * VC}
        wts = {
            (nm, cc): wall_t[cc][:, off : off + VC]
            for nm, off in _woff.items()
            for cc in range(2)
        }

        K_t = persist.tile([128, N], f16, tag="K")
        Q_t = persist.tile([128, MQ], f16, tag="Q")
        VT = persist.tile([128, KC * 128], bf16, tag="VT")

        xk_t = [
            xp.tile([128, N], f16, tag=f"xk{cc}", name=f"xk{cc}") for cc in range(2)
        ]
        xq_t = [
            xp.tile([128, MQ], f16, tag=f"xq{cc}", name=f"xq{cc}") for cc in range(2)
        ]

        # Input DMA: each packed block is striped cc0->sync / cc1->gpsimd so
        # both halves complete together at the combined HBM rate, in
        # consumption order. Per-engine dependency windows bound in-flight
        # pieces so early pieces finish early.
        from concourse.tile_rust import add_dep_helper

        _dmas = {}
        _eng = ((nc.sync, "s"), (nc.gpsimd, "g"))

        def _issue(cc, dst_ap, src_ap):
            eng, ename = _eng[cc]
            lst = _dmas.setdefault(ename, [])
            ins = eng.dma_start(dst_ap, src_ap)
            if len(lst) >= 4:
                add_dep_helper(ins.ins, lst[-4].ins, reason="dma window")
            lst.append(ins)

        for cc in range(2):
            _issue(cc, wall_t[cc][:], wz_d[cc])
        for cc in range(2):
            _issue(cc, xq_t[cc][:, 0:512], xq_d[cc])
        for cc in range(2):
            _issue(cc, xk_t[cc][:, 0:512], xk1_d[cc])
        for cc in range(2):
            _issue(cc, xq_t[cc][:, 512:1024], xq_d[2 + cc])
        for cc in range(2):
            _issue(cc, xk_t[cc][:, 512:1024], xk1_d[2 + cc])
        for blk in range(3):
            for cc in range(2):
                _issue(
                    cc,
                    xk_t[cc][:, 1024 + blk * 1024 : 2048 + blk * 1024],
                    xk2_d[2 * blk + cc],
                )
        for cc in range(2):
            _issue(cc, xq_t[cc][:, 1024:2048], xq2_d[cc])

        def emit_proj_tile(pool, dst, wnm, xt, t, copy_eng=None):
            ps = pool.tile([128, 512], f32, tag="projps", name="ps")
            for cc in range(2):
                nc.tensor.matmul(
                    ps[:],
                    wts[(wnm, cc)][:],
                    xt[cc][:, t * 512 : (t + 1) * 512],
                    start=(cc == 0),
                    stop=(cc == 1),
                )
            dstap = dst[:, t * 512 : (t + 1) * 512]
            if copy_eng is None:
                nc.vector.tensor_copy(dstap, ps[:])
            else:
                copy_eng.copy(dstap, ps[:])

        def emit_vt_quad(pool, q, copy_eng=None):
            # V^T blocks 4q..4q+3: each [n=128, d=128] = x_block.T @ Wv.T;
            # four blocks share one PSUM tile so one wide copy drains them.
            tp = pool.tile([128, 512], f32, tag="projps", name="tp")
            for jj in range(4):
                j = 4 * q + jj
                for cc in range(2):
                    nc.tensor.matmul(
                        tp[:, jj * 128 : (jj + 1) * 128],
                        xk_t[cc][:, j * 128 : (j + 1) * 128],
                        wts[("wv", cc)][:],
                        start=(cc == 0),
                        stop=(cc == 1),
                    )
            if copy_eng is None:
                nc.vector.tensor_copy(VT[:, q * 512 : (q + 1) * 512], tp[:])
            else:
                copy_eng.copy(VT[:, q * 512 : (q + 1) * 512], tp[:])

        # Floors (ms): don't emit projection work before its DMA piece can
        # have landed (input streams at ~0.24 MB/us from ~7.6us), so the
        # in-order PE queue never blocks on a DMA semaphore.
        QF0, KF0, QF1 = 0.0094, 0.0105, 0.0115
        K_FLOOR = {1: 0.0128, 2: 0.0148, 3: 0.0152, 4: 0.0168, 5: 0.0172,
                   6: 0.0188, 7: 0.0192}
        V_FLOOR = {0: 0.0120, 1: 0.0130, 2: 0.0150, 3: 0.0154, 4: 0.0170,
                   5: 0.0174, 6: 0.0190, 7: 0.0194}
        XQ1 = 0.0208

        spool = ctx.enter_context(tc.tile_pool(name="spool", bufs=2, space="PSUM"))
        pcpool = ctx.enter_context(tc.tile_pool(name="pcpool", bufs=1, space="PSUM"))

        with tc.tile_pool(name="projps", bufs=2, space="PSUM") as pps:
            # K/VT copies ride the idle ACT engine pre-first-exp so the
            # QK(0,0) critical path never waits behind DVE copy order.
            with tc.tile_wait_until(QF0):
                emit_proj_tile(pps, Q_t, "wq", xq_t, 0)
            with tc.tile_wait_until(KF0):
                emit_proj_tile(pps, K_t, "wk", xk_t, 0, copy_eng=nc.scalar)
            with tc.tile_wait_until(QF1):
                emit_proj_tile(pps, Q_t, "wq", xq_t, 1)

        with (
            tc.tile_pool(name="lzps", bufs=2, space="PSUM") as lzps,
            tc.tile_pool(name="epool", bufs=4) as epool,
            tc.tile_pool(name="treep", bufs=2) as treep,
            tc.tile_pool(name="fold", bufs=3) as foldp,
            tc.tile_pool(name="opool", bufs=2) as opool,
        ):
            pairs = [(qt, j) for qt in range(2) for j in range(KC)]
            ps_tiles = {}

            def emit_qk(qt, j):
                ps = spool.tile([128, QT], f32, tag="ps", name="ps")
                for qq in range(2):
                    nc.tensor.matmul(
                        ps[:, qq * 512 : (qq + 1) * 512],
                        K_t[:, j * 128 : (j + 1) * 128],
                        Q_t[:, qt * QT + qq * 512 : qt * QT + (qq + 1) * 512],
                        start=True,
                        stop=True,
                    )
                ps_tiles[(qt, j)] = ps

            pc = None
            es_dt = None
            acc = None
            nfold = None
            emit_qk(*pairs[0])
            for i, (qt, j) in enumerate(pairs):
                if i + 1 < len(pairs):
                    emit_qk(*pairs[i + 1])
                if j == 0:
                    pc = pcpool.tile([128, QT], f32, tag="pc", name="pc")
                ps = ps_tiles.pop((qt, j))
                if j % 2 == 0:
                    es_dt = epool.tile([128, 2 * QT], bf16, tag="es", name="es")
                es = es_dt[:, (j % 2) * QT : (j % 2 + 1) * QT]
                nc.scalar.activation(es, ps[:], Exp)
                # interleave remaining projections into the first pass,
                # one per two iterations so the PE never starves ACT (each
                # K tile t / VT quad q is consumed from iteration 4t / 4q)
                if qt == 0 and j % 2 == 1 and j <= 27:
                    s = (j + 1) // 2       # 1..14
                    if s == 0:
                        pass
                    elif s % 2 == 1:
                        t = (s + 1) // 2   # K tiles 1..7 at j=1,5,...
                        with tc.tile_wait_until(K_FLOOR[t]):
                            emit_proj_tile(lzps, K_t, "wk", xk_t, t)
                    else:
                        q = s // 2         # VT quads 1..7 at j=3,7,...
                        with tc.tile_wait_until(V_FLOOR[q]):
                            emit_vt_quad(lzps, q)
                if qt == 0 and j == 0:
                    with tc.tile_wait_until(V_FLOOR[0]):
                        emit_vt_quad(lzps, 0, copy_eng=nc.scalar)
                if qt == 0 and j in (26, 28):
                    with tc.tile_wait_until(XQ1):
                        emit_proj_tile(lzps, Q_t, "wq", xq_t, 2 + (j - 26) // 2)
                first, last = j == 0, j == KC - 1
                for qq in range(2):
                    sl = slice(qq * 512, (qq + 1) * 512)
                    nc.tensor.matmul(
                        pc[:, sl],
                        VT[:, j * 128 : (j + 1) * 128],
                        es[:, qq * 512 : (qq + 1) * 512],
                        start=first,
                        stop=last,
                    )
                if last:
                    # oc drain first in queue order so pc frees ASAP for the
                    # next qt pass; qt1's copies split ACT/DVE to parallelize
                    # the tail; DMAs split across both queues.
                    so = opool.tile([128, QT], f32, tag="so", name="so")
                    for qq in range(2):
                        sl = slice(qq * 512, (qq + 1) * 512)
                        if qt == 0 or qq == 1:
                            nc.vector.tensor_copy(so[:, sl], pc[:, sl])
                        else:
                            nc.scalar.copy(so[:, sl], pc[:, sl])
                        eng = nc.gpsimd if qq == 0 else nc.sync
                        eng.dma_start(oc_d[qt, :, sl], so[:, sl])
                # Softmax partial sums on DVE. Groups 0-2 (j 0..23): wide
                # [128, 2048] chained adds over 4 double-tiles, then one fold
                # to [128, QT]. Group 3 (j 24..31): per-double-tile narrow
                # folds chained, with the last fold shipped as its own
                # partial so the post-last-exp tail is just fold+DMA.
                if j % 2 == 1:
                    g, m = j // 8, (j % 8) // 2
                    if g < 3:
                        if m == 0:
                            acc = es_dt
                        else:
                            if m == 1:
                                nacc = treep.tile(
                                    [128, 2 * QT], bf16, tag="acc", name="acc"
                                )
                                nc.vector.tensor_add(nacc[:], acc[:], es_dt[:])
                                acc = nacc
                            else:
                                nc.vector.tensor_add(acc[:], acc[:], es_dt[:])
                            if m == 3:
                                fo = foldp.tile(
                                    [128, QT], bf16, tag="fo", name="fo"
                                )
                                nc.vector.tensor_add(
                                    fo[:], acc[:, 0:QT], acc[:, QT : 2 * QT]
                                )
                                nc.sync.dma_start(oss_d[qt, g], fo[:])
                    else:
                        fo = foldp.tile([128, QT], bf16, tag="fo", name="fo")
                        nc.vector.tensor_add(
                            fo[:], es_dt[:, 0:QT], es_dt[:, QT : 2 * QT]
                        )
                        if m == 0:
                            nfold = fo
                        elif m < 3:
                            nc.vector.tensor_add(nfold[:], nfold[:], fo[:])
                            if m == 2:
                                nc.sync.dma_start(oss_d[qt, 3], nfold[:])
                        else:
                            nc.sync.dma_start(oss_d[qt, 4], fo[:])
    nc.compile()
    return nc


def make_in_maps(x, Wq, Wk, Wv):
    x = np.asarray(x, dtype=np.float32).reshape(B, C, N).astype(np.float16)
    wz = np.concatenate(
        [np.asarray(w, np.float32).T for w in (Wq, Wk, Wv)], axis=1
    ).astype(np.float16)
    wt = {"wz": np.ascontiguousarray(wz.reshape(2, 128, 3 * VC))}

    def blocks(m, spans):
        # pack [cc-half, col-span] blocks of a [C, *] matrix contiguously
        return np.stack(
            [m[cc * 128 : (cc + 1) * 128, c0:c1] for c0, c1 in spans for cc in (0, 1)]
        )

    in_maps = []
    for core in range(8):
        b, h = core // 2, core % 2
        xc = x[b]
        xq = xc[:, h * MQ : (h + 1) * MQ]
        in_maps.append(
            {
                "xq": blocks(xq, [(0, 512), (512, 1024)]),
                "xq2": blocks(xq, [(1024, 2048)]),
                "xk1": blocks(xc, [(0, 512), (512, 1024)]),
                "xk2": blocks(
                    xc, [(1024, 2048), (2048, 3072), (3072, 4096)]
                ),
                **wt,
            }
        )
    return in_maps


def assemble_output(results):
    out = np.empty((B, VC, N), dtype=np.float32)
    for core, r in enumerate(results):
        b, h = core // 2, core % 2
        # oss: [2, NG, 128, QT] partial sums; reduce groups+partitions
        sums = r["oss"].astype(np.float32).sum(axis=(1, 2))[:, None, :]  # [2,1,QT]
        core_out = r["oc"] / sums                                        # [2,128,QT]
        out[b, :, h * MQ : (h + 1) * MQ] = np.concatenate(
            [core_out[0], core_out[1]], axis=1
        )
    return out.reshape(B, VC, H, W)


def _results_sane(results):
    for r in results:
        oc, oss = r["oc"], np.asarray(r["oss"], dtype=np.float32)
        if not (np.isfinite(oc).all() and np.isfinite(oss).all()):
            return False
        if oss.sum(axis=(1, 2)).min() <= 0.0:      # softmax denominators
            return False
    return True


def kernel(x, Wq, Wk, Wv):
    global _cached_nc
    from concourse.bass_utils import run_bass_kernel_spmd

    if _cached_nc is None:
        _cached_nc = _build()
    in_maps = make_in_maps(x, Wq, Wk, Wv)
    results = None
    for attempt in range(3):
        try:
            res = run_bass_kernel_spmd(
                _cached_nc, in_maps, core_ids=list(range(8))
            )
        except Exception:
            if attempt == 2:
                raise
            continue
        results = res.results
        if _results_sane(results):
            break
    return assemble_output(results)
